# revision 6
# baseline (speedup 1.0000x reference)
"""Trainium2 Bass kernel for nn_DeconvBlock (dynamic-weight transposed conv).

Computes, per sample b:
    w_b   = weight + sum_j feature[b,j] * (t_j * m_j)            (weight synthesis)
    out_b = conv_transpose2d(x_b, w_b, stride=2, pad=1, K=4)     (grouped over batch)
    out   = prelu(out_b + bias, a)

Strategy (data-parallel over batch, 8 cores x 2 samples):
  - conv_transpose(stride 2, K=4, P=1) decomposes into 4 output phases
    (py,px) in {0,1}^2; each phase output pixel is a sum of 4 "taps"
    (ky,kx), each tap a 1x1 conv (matmul over CIN=256) of a +-1 shifted x.
  - Operands are fp16 (PE streams 16-bit moving operands at 1 col/cycle;
    accumulation stays fp32 in PSUM; measured rel err ~4e-4). 512 matmuls
    /core of [128x128] @ [128x512] ~= 109us = the PE streaming roofline
    for the 4.3 GMAC/core workload; the kernel is structured to keep that
    stream dense and to minimize time outside it.
  - Per-sample weight synthesis (0.2% of the FLOPs) happens on the host
    as part of input layout prep: one (B,4)x(4,CIN*COUT*K*K) sgemm. The
    device receives ready per-sample weights, phase-grouped so the first
    256KB weight DMA unblocks the first matmul.
  - Startup: weight DMAs issue on the sync HWDGE queue while x DMAs
    issue concurrently on the scalar HWDGE queue (issue cost per
    dma_start is ~0.6us of sequencer time, so the two queues halve it).
    Scratch warm-up matmuls run during the DMA wait so the PE HAM clock
    gate is already at 2.4 GHz when the real stream starts.
  - Epilogue: ScalarE adds bias (Identity activation w/ per-partition
    bias), VectorE computes prelu(t) = max(t, a*t) in one fused op while
    interleaving the 4 phases into contiguous output rows; each row
    block then leaves in ONE contiguous DMA (8KB/partition descriptors
    instead of 512B strided rows -> 16x fewer DMA descriptors).
"""

import numpy as np

import concourse.bass as bass
import concourse.mybir as mybir
from concourse import bacc
from concourse import bass_utils
from concourse.tile import TileContext

B, CIN, COUT, H, W, K, S = 16, 256, 128, 64, 64, 4, 2
NCORES = 8
BPC = B // NCORES  # samples per core
P = 128
NCH = CIN // P     # ic chunks of 128
HP = H + 2         # padded x height/width (zero border of 1)
NROW = 8           # output-phase rows per block
NYB = H // NROW    # row blocks per sample
NWARM = 22         # PE warm-up matmuls (~2.4us at cold clock)

# phase py -> ((ky, sy), ...): contribution x[y'+sy] * w[ky]
_TAPS = {0: ((1, 0), (3, -1)), 1: ((2, 0), (0, 1))}

_COMPILED = None


def _build():
    f32 = mybir.dt.float32
    f16 = mybir.dt.float16
    Alu = mybir.AluOpType
    Act = mybir.ActivationFunctionType

    nc = bacc.Bacc(
        "TRN2", target_bir_lowering=False, debug=False, num_devices=NCORES
    )
    x_d = nc.dram_tensor(
        "x_sh", (BPC, NCH, P, HP, HP), f16, kind="ExternalInput"
    ).ap()
    # host-synthesized per-sample weights, phase-grouped:
    # (cin_part, sample, phase, chunk, iy, ix, cout)
    w_d = nc.dram_tensor(
        "wph", (P, BPC, 4, NCH, 2, 2, COUT), f16, kind="ExternalInput"
    ).ap()
    bias_d = nc.dram_tensor("biasb", (P, 1), f32, kind="ExternalInput").ap()
    a_d = nc.dram_tensor("ab", (P, 1), f32, kind="ExternalInput").ap()
    out_d = nc.dram_tensor(
        "out_sh", (BPC, COUT, H * S, W * S), f32, kind="ExternalOutput"
    ).ap()

    with TileContext(nc) as tc:
        with (
            tc.tile_pool(name="const", bufs=1) as const_pool,
            tc.tile_pool(name="w_pool", bufs=1) as w_pool,
            tc.tile_pool(name="x_pool", bufs=1) as x_pool,
            tc.tile_pool(name="t_pool", bufs=6) as t_pool,
            tc.tile_pool(name="row_pool", bufs=4) as row_pool,
            tc.tile_pool(name="psum", bufs=8, space="PSUM") as psum_pool,
        ):
            bias_t = const_pool.tile([P, 1], f32)
            a_t = const_pool.tile([P, 1], f32)
            warm_t = const_pool.tile([P, P], f16)
            scratch_t = const_pool.tile([P, 1], f32)

            wt = []
            xt = []
            for s in range(BPC):
                w_s = w_pool.tile(
                    [P, 4, NCH, 2, 2, COUT], f16, name=f"wt{s}", tag=f"wt{s}"
                )
                wt.append(w_s)
                x_s = x_pool.tile(
                    [P, NCH, HP, HP], f16, name=f"xpad{s}", tag=f"xpad{s}"
                )
                xt.append(x_s)

            # ---- startup DMAs ----
            # weights on the sync HWDGE queue; x on the scalar HWDGE queue
            # (parallel issue).  Order == priority within each queue.  The
            # first row-block needs only w phase 0 + x rows 0:10 (594KB) —
            # everything else is issued behind that critical set.
            nc.sync.dma_start(wt[0][:, 0], w_d[:, 0, 0])
            nc.scalar.dma_start(xt[0][:, 0, 0:10], x_d[0, 0, :, 0:10])
            nc.scalar.dma_start(xt[0][:, 1, 0:10], x_d[0, 1, :, 0:10])
            for ph in range(1, 4):
                nc.sync.dma_start(wt[0][:, ph], w_d[:, 0, ph])
            nc.scalar.dma_start(xt[0][:, 0, 10:34], x_d[0, 0, :, 10:34])
            nc.scalar.dma_start(xt[0][:, 1, 10:34], x_d[0, 1, :, 10:34])
            nc.sync.dma_start(bias_t[:], bias_d[:])
            nc.sync.dma_start(a_t[:], a_d[:])
            nc.scalar.dma_start(xt[0][:, 0, 34:HP], x_d[0, 0, :, 34:HP])
            nc.scalar.dma_start(xt[0][:, 1, 34:HP], x_d[0, 1, :, 34:HP])
            for ph in range(4):
                nc.sync.dma_start(wt[1][:, ph], w_d[:, 1, ph])
            nc.scalar.dma_start(xt[1][:, 0], x_d[1, 0])
            nc.scalar.dma_start(xt[1][:, 1], x_d[1, 1])

            # warm the ScalarE activation table (Identity) during DMA wait
            nc.vector.memset(scratch_t[:], 0.0)
            nc.scalar.activation(scratch_t[:], scratch_t[:], Act.Identity, scale=1.0)

            # ---- PE warm-up: scratch matmuls while weights stream in ----
            nc.vector.memset(warm_t[:], 0.0)
            for i in range(NWARM):
                ps_w = psum_pool.tile([P, NROW, W], f32, name="ps", tag="ps")
                nc.tensor.matmul(
                    ps_w[:, 0:2, :], warm_t[:], warm_t[:], start=True, stop=True
                )

            # ---- main conv loop ----
            # The final sample's last block is split into two 4-row halves
            # so the first half's output DMA overlaps the second half's
            # matmuls, shortening the post-stream drain.
            blocks = [(NROW * i, NROW) for i in range(NYB)]
            last_blocks = blocks[:-1] + [
                (NROW * (NYB - 1), 4),
                (NROW * (NYB - 1) + 4, 4),
            ]
            for s in range(BPC):
                for by0, nr in last_blocks if s == BPC - 1 else blocks:
                    # row_t free layout (y', py, x', px) == out rows
                    # [2*nr, 2*W] for oy in [2*by0, 2*(by0+nr))
                    row_t = row_pool.tile(
                        [P, nr, 2, W, 2], f32, name="row_t", tag="row_t"
                    )
                    for py in (0, 1):
                        for px in (0, 1):
                            ph = 2 * py + px
                            ps = psum_pool.tile(
                                [P, nr, W], f32, name="ps", tag="ps"
                            )
                            k = 0
                            for c in range(NCH):
                                for iy, (ky, sy) in enumerate(_TAPS[py]):
                                    for ix, (kx, sx) in enumerate(_TAPS[px]):
                                        lhsT = wt[s][:, ph, c, iy, ix, :]
                                        y0 = 1 + sy + by0
                                        x0 = 1 + sx
                                        rhs = xt[s][
                                            :, c, y0 : y0 + nr, x0 : x0 + W
                                        ]
                                        nc.tensor.matmul(
                                            ps[:],
                                            lhsT,
                                            rhs,
                                            start=(k == 0),
                                            stop=(k == 7),
                                        )
                                        k += 1
                            tt = t_pool.tile([P, nr, W], f32, name="tt", tag="tt")
                            nc.scalar.activation(
                                tt[:], ps[:], Act.Identity, bias=bias_t[:], scale=1.0
                            )
                            # prelu(t) = max(t, a*t), interleaved into row_t
                            nc.vector.scalar_tensor_tensor(
                                row_t[:, :, py, :, px],
                                tt[:],
                                a_t[:],
                                tt[:],
                                op0=Alu.mult,
                                op1=Alu.max,
                            )
                    # one contiguous DMA per block: rows 2*by0 .. 2*by0+2*nr
                    nc.sync.dma_start(
                        out_d[s, :, 2 * by0 : 2 * (by0 + nr), :],
                        row_t[:],
                    )

    nc.compile()
    return nc


def _get_compiled():
    global _COMPILED
    if _COMPILED is None:
        _COMPILED = _build()
    return _COMPILED


# host-side tap gather indices: KY[ph,iy,ix], KX[ph,iy,ix]
_KG = np.array([[1, 3], [2, 0]])  # [p, i] -> k index
_KY = np.zeros((4, 2, 2), np.intp)
_KX = np.zeros((4, 2, 2), np.intp)
for _py in range(2):
    for _px in range(2):
        for _iy in range(2):
            for _ix in range(2):
                _KY[2 * _py + _px, _iy, _ix] = _KG[_py, _iy]
                _KX[2 * _py + _px, _iy, _ix] = _KG[_px, _ix]


def _prep_in_maps(inputs):
    x = np.asarray(inputs["x"], dtype=np.float32)
    xp = np.zeros((B, NCH, P, HP, HP), dtype=np.float16)
    xp[:, :, :, 1 : HP - 1, 1 : HP - 1] = x.reshape(B, NCH, P, H, W)

    # per-sample weight synthesis: one (B,4) @ (4, CIN*COUT*K*K) sgemm
    feat = np.asarray(inputs["feature"], dtype=np.float32)
    w = np.asarray(inputs["weight"], dtype=np.float32)
    tm = np.stack(
        [
            np.asarray(inputs[f"t_{n}"], dtype=np.float32)[0]
            * np.asarray(inputs[f"m_{n}"], dtype=np.float32)[0]
            for n in ("bayer", "quad", "nano", "qxq")
        ]
    )  # (4, CIN, COUT, K, K)
    wb = (feat @ tm.reshape(4, -1)).reshape(B, CIN, COUT, K, K)
    wb += w[None]
    # phase-grouped gather: (B, NCH, P, COUT, 4, 2, 2) -> (P,B,4,NCH,2,2,COUT)
    wr = wb.reshape(B, NCH, P, COUT, K, K)
    wsel = wr[:, :, :, :, _KY, _KX]  # (B, NCH, P, COUT, 4, 2, 2)
    wph = np.ascontiguousarray(
        wsel.transpose(2, 0, 4, 1, 5, 6, 3), dtype=np.float16
    )  # (P, B, 4, NCH, 2, 2, COUT)

    biasb = np.ascontiguousarray(
        np.asarray(inputs["bias"], dtype=np.float32).reshape(P, 1)
    )
    ab = np.ascontiguousarray(
        np.broadcast_to(
            np.asarray(inputs["prelu_a"], dtype=np.float32).reshape(1, 1), (P, 1)
        )
    )
    in_maps = []
    for i in range(NCORES):
        sl = slice(i * BPC, (i + 1) * BPC)
        in_maps.append(
            {
                "x_sh": xp[sl],
                "wph": np.ascontiguousarray(wph[:, sl]),
                "biasb": biasb,
                "ab": ab,
            }
        )
    return in_maps


def kernel(**inputs):
    nc = _get_compiled()
    in_maps = _prep_in_maps(inputs)
    res = bass_utils.run_bass_kernel_spmd(nc, in_maps, core_ids=list(range(NCORES)))
    return np.concatenate(
        [res.results[i]["out_sh"] for i in range(NCORES)], axis=0
    )


# revision 8
# speedup vs baseline: 1.0250x; 1.0250x over previous
"""Trainium2 Bass kernel for nn_DeconvBlock (dynamic-weight transposed conv).

Computes, per sample b:
    w_b   = weight + sum_j feature[b,j] * (t_j * m_j)            (weight synthesis)
    out_b = conv_transpose2d(x_b, w_b, stride=2, pad=1, K=4)     (grouped over batch)
    out   = prelu(out_b + bias, a)

Strategy (data-parallel over batch, 8 cores x 2 samples):
  - conv_transpose(stride 2, K=4, P=1) decomposes into 4 output phases
    (py,px) in {0,1}^2; each phase output pixel is a sum of 4 "taps"
    (ky,kx), each tap a 1x1 conv (matmul over CIN=256) of a +-1 shifted x.
  - Operands are fp16 (PE streams 16-bit moving operands at 1 col/cycle;
    accumulation stays fp32 in PSUM; measured rel err ~4e-4). 512 matmuls
    /core of [128x128] @ [128x512] ~= 109us = the PE streaming roofline
    for the 4.3 GMAC/core workload; the kernel is structured to keep that
    stream dense and to minimize time outside it.
  - Per-sample weight synthesis (0.2% of the FLOPs) happens on the host
    as part of input layout prep: one (B,4)x(4,CIN*COUT*K*K) sgemm. The
    device receives ready per-sample weights, phase-grouped so the first
    256KB weight DMA unblocks the first matmul.
  - Startup: weight DMAs issue on the sync HWDGE queue while x DMAs
    issue concurrently on the scalar HWDGE queue (issue cost per
    dma_start is ~0.6us of sequencer time, so the two queues halve it).
    Scratch warm-up matmuls run during the DMA wait so the PE HAM clock
    gate is already at 2.4 GHz when the real stream starts.
  - Epilogue: ScalarE adds bias (Identity activation w/ per-partition
    bias), VectorE computes prelu(t) = max(t, a*t) in one fused op while
    interleaving the 4 phases into contiguous output rows; each row
    block then leaves in ONE contiguous DMA (8KB/partition descriptors
    instead of 512B strided rows -> 16x fewer DMA descriptors).
"""

import numpy as np

import concourse.bass as bass
import concourse.mybir as mybir
from concourse import bacc
from concourse import bass_utils
from concourse.tile import TileContext

B, CIN, COUT, H, W, K, S = 16, 256, 128, 64, 64, 4, 2
NCORES = 8
BPC = B // NCORES  # samples per core
P = 128
NCH = CIN // P     # ic chunks of 128
HP = H + 2         # padded x height/width (zero border of 1)
NROW = 8           # output-phase rows per block
NYB = H // NROW    # row blocks per sample
NWARM = 50         # PE warm-up matmuls: bridge the ~5us DMA wait so the
                   # HAM clock gate is warm when the real stream starts

# phase py -> ((ky, sy), ...): contribution x[y'+sy] * w[ky]
_TAPS = {0: ((1, 0), (3, -1)), 1: ((2, 0), (0, 1))}

_COMPILED = None


def _build():
    f32 = mybir.dt.float32
    f16 = mybir.dt.float16
    Alu = mybir.AluOpType
    Act = mybir.ActivationFunctionType

    nc = bacc.Bacc(
        "TRN2", target_bir_lowering=False, debug=False, num_devices=NCORES
    )
    x_d = nc.dram_tensor(
        "x_sh", (BPC, NCH, P, HP, HP), f16, kind="ExternalInput"
    ).ap()
    # host-synthesized per-sample weights, phase-grouped:
    # (cin_part, sample, phase, chunk, iy, ix, cout)
    w_d = nc.dram_tensor(
        "wph", (P, BPC, 4, NCH, 2, 2, COUT), f16, kind="ExternalInput"
    ).ap()
    bias_d = nc.dram_tensor("biasb", (P, 1), f32, kind="ExternalInput").ap()
    a_d = nc.dram_tensor("ab", (P, 1), f32, kind="ExternalInput").ap()
    out_d = nc.dram_tensor(
        "out_sh", (BPC, COUT, H * S, W * S), f32, kind="ExternalOutput"
    ).ap()

    with TileContext(nc) as tc:
        with (
            tc.tile_pool(name="const", bufs=1) as const_pool,
            tc.tile_pool(name="w_pool", bufs=1) as w_pool,
            tc.tile_pool(name="x_pool", bufs=1) as x_pool,
            tc.tile_pool(name="t_pool", bufs=6) as t_pool,
            tc.tile_pool(name="row_pool", bufs=4) as row_pool,
            tc.tile_pool(name="psum", bufs=8, space="PSUM") as psum_pool,
        ):
            bias_t = const_pool.tile([P, 1], f32)
            a_t = const_pool.tile([P, 1], f32)
            warm_t = const_pool.tile([P, P], f16)
            scratch_t = const_pool.tile([P, 1], f32)

            wt = []
            xt = []
            for s in range(BPC):
                w_s = w_pool.tile(
                    [P, 4, NCH, 2, 2, COUT], f16, name=f"wt{s}", tag=f"wt{s}"
                )
                wt.append(w_s)
                x_s = x_pool.tile(
                    [P, NCH, HP, HP], f16, name=f"xpad{s}", tag=f"xpad{s}"
                )
                xt.append(x_s)

            # ---- startup DMAs ----
            # weights on the sync HWDGE queue; x on the scalar HWDGE queue
            # (parallel issue).  Order == priority within each queue.  The
            # first row-block needs only w phase 0 + x rows 0:10 (594KB) —
            # everything else is issued behind that critical set.
            nc.sync.dma_start(wt[0][:, 0], w_d[:, 0, 0])
            nc.scalar.dma_start(xt[0][:, 0, 0:10], x_d[0, 0, :, 0:10])
            nc.scalar.dma_start(xt[0][:, 1, 0:10], x_d[0, 1, :, 0:10])
            for ph in range(1, 4):
                nc.sync.dma_start(wt[0][:, ph], w_d[:, 0, ph])
            nc.scalar.dma_start(xt[0][:, 0, 10:34], x_d[0, 0, :, 10:34])
            nc.scalar.dma_start(xt[0][:, 1, 10:34], x_d[0, 1, :, 10:34])
            nc.sync.dma_start(bias_t[:], bias_d[:])
            nc.sync.dma_start(a_t[:], a_d[:])
            nc.scalar.dma_start(xt[0][:, 0, 34:HP], x_d[0, 0, :, 34:HP])
            nc.scalar.dma_start(xt[0][:, 1, 34:HP], x_d[0, 1, :, 34:HP])
            for ph in range(4):
                nc.sync.dma_start(wt[1][:, ph], w_d[:, 1, ph])
            nc.scalar.dma_start(xt[1][:, 0], x_d[1, 0])
            nc.scalar.dma_start(xt[1][:, 1], x_d[1, 1])

            # warm the ScalarE activation table (Identity) during DMA wait
            nc.vector.memset(scratch_t[:], 0.0)
            nc.scalar.activation(scratch_t[:], scratch_t[:], Act.Identity, scale=1.0)

            # ---- PE warm-up: scratch matmuls while weights stream in ----
            nc.vector.memset(warm_t[:], 0.0)
            for i in range(NWARM):
                ps_w = psum_pool.tile([P, NROW, W], f32, name="ps", tag="ps")
                nc.tensor.matmul(
                    ps_w[:, 0:2, :], warm_t[:], warm_t[:], start=True, stop=True
                )

            # ---- main conv loop ----
            # The final sample's last block is split into two 4-row halves
            # so the first half's output DMA overlaps the second half's
            # matmuls, shortening the post-stream drain.
            blocks = [(NROW * i, NROW) for i in range(NYB)]
            last_blocks = blocks[:-1] + [
                (NROW * (NYB - 1), 4),
                (NROW * (NYB - 1) + 4, 4),
            ]
            for s in range(BPC):
                for by0, nr in last_blocks if s == BPC - 1 else blocks:
                    # row_t free layout (y', py, x', px) == out rows
                    # [2*nr, 2*W] for oy in [2*by0, 2*(by0+nr))
                    row_t = row_pool.tile(
                        [P, nr, 2, W, 2], f32, name="row_t", tag="row_t"
                    )
                    for py in (0, 1):
                        for px in (0, 1):
                            ph = 2 * py + px
                            ps = psum_pool.tile(
                                [P, nr, W], f32, name="ps", tag="ps"
                            )
                            k = 0
                            for c in range(NCH):
                                for iy, (ky, sy) in enumerate(_TAPS[py]):
                                    for ix, (kx, sx) in enumerate(_TAPS[px]):
                                        lhsT = wt[s][:, ph, c, iy, ix, :]
                                        y0 = 1 + sy + by0
                                        x0 = 1 + sx
                                        rhs = xt[s][
                                            :, c, y0 : y0 + nr, x0 : x0 + W
                                        ]
                                        nc.tensor.matmul(
                                            ps[:],
                                            lhsT,
                                            rhs,
                                            start=(k == 0),
                                            stop=(k == 7),
                                        )
                                        k += 1
                            tt = t_pool.tile([P, nr, W], f32, name="tt", tag="tt")
                            nc.scalar.activation(
                                tt[:], ps[:], Act.Identity, bias=bias_t[:], scale=1.0
                            )
                            # prelu(t) = max(t, a*t), interleaved into row_t
                            nc.vector.scalar_tensor_tensor(
                                row_t[:, :, py, :, px],
                                tt[:],
                                a_t[:],
                                tt[:],
                                op0=Alu.mult,
                                op1=Alu.max,
                            )
                    # one contiguous DMA per block: rows 2*by0 .. 2*by0+2*nr
                    # (alternate HWDGE queues; split the final block across
                    # both so its descriptor generation runs in parallel)
                    last = s == BPC - 1 and by0 + nr == H
                    if last:
                        h = nr // 2
                        nc.sync.dma_start(
                            out_d[s, :, 2 * by0 : 2 * (by0 + h), :],
                            row_t[:, 0:h],
                        )
                        nc.scalar.dma_start(
                            out_d[s, :, 2 * (by0 + h) : 2 * (by0 + nr), :],
                            row_t[:, h:nr],
                        )
                    elif (by0 // NROW) % 2 == 0:
                        nc.sync.dma_start(
                            out_d[s, :, 2 * by0 : 2 * (by0 + nr), :],
                            row_t[:],
                        )
                    else:
                        nc.scalar.dma_start(
                            out_d[s, :, 2 * by0 : 2 * (by0 + nr), :],
                            row_t[:],
                        )

    nc.compile()
    return nc


def _get_compiled():
    global _COMPILED
    if _COMPILED is None:
        _COMPILED = _build()
    return _COMPILED


# host-side tap gather indices: KY[ph,iy,ix], KX[ph,iy,ix]
_KG = np.array([[1, 3], [2, 0]])  # [p, i] -> k index
_KY = np.zeros((4, 2, 2), np.intp)
_KX = np.zeros((4, 2, 2), np.intp)
for _py in range(2):
    for _px in range(2):
        for _iy in range(2):
            for _ix in range(2):
                _KY[2 * _py + _px, _iy, _ix] = _KG[_py, _iy]
                _KX[2 * _py + _px, _iy, _ix] = _KG[_px, _ix]


def _prep_in_maps(inputs):
    x = np.asarray(inputs["x"], dtype=np.float32)
    xp = np.zeros((B, NCH, P, HP, HP), dtype=np.float16)
    xp[:, :, :, 1 : HP - 1, 1 : HP - 1] = x.reshape(B, NCH, P, H, W)

    # per-sample weight synthesis: one (B,4) @ (4, CIN*COUT*K*K) sgemm
    feat = np.asarray(inputs["feature"], dtype=np.float32)
    w = np.asarray(inputs["weight"], dtype=np.float32)
    tm = np.stack(
        [
            np.asarray(inputs[f"t_{n}"], dtype=np.float32)[0]
            * np.asarray(inputs[f"m_{n}"], dtype=np.float32)[0]
            for n in ("bayer", "quad", "nano", "qxq")
        ]
    )  # (4, CIN, COUT, K, K)
    wb = (feat @ tm.reshape(4, -1)).reshape(B, CIN, COUT, K, K)
    wb += w[None]
    # phase-grouped gather: (B, NCH, P, COUT, 4, 2, 2) -> (P,B,4,NCH,2,2,COUT)
    wr = wb.reshape(B, NCH, P, COUT, K, K)
    wsel = wr[:, :, :, :, _KY, _KX]  # (B, NCH, P, COUT, 4, 2, 2)
    wph = np.ascontiguousarray(
        wsel.transpose(2, 0, 4, 1, 5, 6, 3), dtype=np.float16
    )  # (P, B, 4, NCH, 2, 2, COUT)

    biasb = np.ascontiguousarray(
        np.asarray(inputs["bias"], dtype=np.float32).reshape(P, 1)
    )
    ab = np.ascontiguousarray(
        np.broadcast_to(
            np.asarray(inputs["prelu_a"], dtype=np.float32).reshape(1, 1), (P, 1)
        )
    )
    in_maps = []
    for i in range(NCORES):
        sl = slice(i * BPC, (i + 1) * BPC)
        in_maps.append(
            {
                "x_sh": xp[sl],
                "wph": np.ascontiguousarray(wph[:, sl]),
                "biasb": biasb,
                "ab": ab,
            }
        )
    return in_maps


def kernel(**inputs):
    nc = _get_compiled()
    in_maps = _prep_in_maps(inputs)
    res = bass_utils.run_bass_kernel_spmd(nc, in_maps, core_ids=list(range(NCORES)))
    return np.concatenate(
        [res.results[i]["out_sh"] for i in range(NCORES)], axis=0
    )
